# revision 1
# baseline (speedup 1.0000x reference)
"""Bass/Trainium2 kernel for nn_LSTMRecommender.

Strategy (8 NeuronCores, SPMD):
  - Data-parallel over batch: each core handles 128 of the 1024 rows.
  - On-device per core:
      * indirect-DMA gathers of product/category embeddings (+ L-mean via
        a strided vector reduce, with the 1/20 folded into W_ih0 on host)
      * timestamp / user-feature linears on the vector engine
      * 2-layer LSTM in feature-major (transposed) layout: gates accumulate
        in PSUM, sigmoid/tanh on the scalar engine, cell math on DVE
      * MLP head + fc2 (vocab) projection streamed from HBM
  - Host only reshapes/transposes weights, shards inputs, concatenates the
    per-core logits.

Self-contained: hardcodes all shapes from the problem spec.
"""

import numpy as np
from contextlib import ExitStack

import concourse.bass as bass
import concourse.mybir as mybir
import concourse.tile as tile
from concourse import bacc
from concourse.bass import IndirectOffsetOnAxis
from concourse.masks import make_identity

# ---------------- problem constants ----------------
B, S, L = 1024, 50, 20
NPROD = 100001          # rows of product embedding table (incl. padding row 0)
NCAT = 1001
PD, CD, TD, UD = 64, 32, 16, 16
HID = 128
IN = PD + CD + TD + UD  # 128
NCORES = 8
BL = B // NCORES        # 128 batch rows per core

VTILE = 512             # logits tile width (one PSUM bank of fp32)
NT = 196                # number of vocab tiles: 196*512 = 100352 >= 100001
VP = NT * VTILE         # padded vocab
NPAIR = NT // 2         # 98 pairs (two 64-row tiles stacked into 128 partitions)
CP = 7                  # pairs per streamed W2 chunk -> 14 chunks
NCHUNK = NPAIR // CP

NSB = 2                 # timesteps per gather block
NGB = S // NSB          # 25 gather blocks
GRP = 4                 # LSTM timesteps per group (x-part batching)

F32 = mybir.dt.float32
I32 = mybir.dt.int32

# dtype config (v1: all fp32)
TAB_DT = F32            # embedding tables in HBM
W_DT = F32              # LSTM weights + x/h matmul operands
W2_DT = F32             # fc2 weight stream
OUT_DT = F32            # logits written to HBM

AF = mybir.ActivationFunctionType
ALU = mybir.AluOpType


def _ext(ap, dims):
    """Return a new AP over the same tensor with an explicit [step,count] list."""
    return bass.AP(tensor=ap.tensor, offset=ap.offset, ap=dims)


def build_nc(debug_taps=False):
    nc = bacc.Bacc("TRN2", target_bir_lowering=False, debug=False,
                   enable_asserts=False, num_devices=NCORES)
    if debug_taps:
        xall_t = nc.dram_tensor("dbg_xall", [BL, S, IN], F32,
                                kind="ExternalOutput").ap()
        y0_t = nc.dram_tensor("dbg_y0", [S, HID, BL], F32,
                              kind="ExternalOutput").ap()
        h1_t = nc.dram_tensor("dbg_h1", [HID, BL], F32,
                              kind="ExternalOutput").ap()
        hid_t = nc.dram_tensor("dbg_hid", [HID, BL], F32,
                               kind="ExternalOutput").ap()

    # ---- DRAM I/O ----
    pidx_d = nc.dram_tensor("pidx", [BL, S * L], I32, kind="ExternalInput").ap()
    cidx_d = nc.dram_tensor("cidx", [BL, S * L], I32, kind="ExternalInput").ap()
    tss_d = nc.dram_tensor("tss", [BL, S], F32, kind="ExternalInput").ap()
    ag_d = nc.dram_tensor("ag", [BL, 2], F32, kind="ExternalInput").ap()
    embp_d = nc.dram_tensor("embp", [NPROD, PD], TAB_DT, kind="ExternalInput").ap()
    embc_d = nc.dram_tensor("embc", [NCAT, CD], TAB_DT, kind="ExternalInput").ap()
    wih0_d = nc.dram_tensor("wih0t", [IN, 4 * HID], W_DT, kind="ExternalInput").ap()
    whh0_d = nc.dram_tensor("whh0t", [HID, 4 * HID], W_DT, kind="ExternalInput").ap()
    wih1_d = nc.dram_tensor("wih1t", [HID, 4 * HID], W_DT, kind="ExternalInput").ap()
    whh1_d = nc.dram_tensor("whh1t", [HID, 4 * HID], W_DT, kind="ExternalInput").ap()
    b0_d = nc.dram_tensor("bias0c", [HID, 4], F32, kind="ExternalInput").ap()
    b1g_d = nc.dram_tensor("bias1c", [HID, 4], F32, kind="ExternalInput").ap()
    w1t_d = nc.dram_tensor("w1t", [HID, HID // 2], W_DT, kind="ExternalInput").ap()
    b1_d = nc.dram_tensor("b1c", [HID // 2, 1], F32, kind="ExternalInput").ap()
    wts_d = nc.dram_tensor("wtsrows", [5, TD], F32, kind="ExternalInput").ap()
    w2s_d = nc.dram_tensor("w2s", [128, NPAIR * VTILE], W2_DT, kind="ExternalInput").ap()
    b2r_d = nc.dram_tensor("b2p", [1, VP], F32, kind="ExternalInput").ap()
    out_d = nc.dram_tensor("logits", [BL, VP], OUT_DT, kind="ExternalOutput").ap()

    with tile.TileContext(nc) as tc, ExitStack() as top:
        const = top.enter_context(tc.tile_pool(name="const", bufs=1))
        h1p = top.enter_context(tc.tile_pool(name="h1p", bufs=2))

        # persistent constants
        wih0t = const.tile([IN, 4 * HID], W_DT)
        whh0t = const.tile([HID, 4 * HID], W_DT)
        wih1t = const.tile([HID, 4 * HID], W_DT)
        whh1t = const.tile([HID, 4 * HID], W_DT)
        bias0c = const.tile([HID, 4], F32)
        bias1c = const.tile([HID, 4], F32)
        w1t = const.tile([HID, HID // 2], W_DT)
        b1c = const.tile([HID // 2, 1], F32)
        wtsr = const.tile([BL, 5, TD], F32)  # replicated across partitions
        ones = const.tile([1, BL], W2_DT)
        ident = const.tile([128, 128], W_DT)
        for sb, dr in ((wih0t, wih0_d), (whh0t, whh0_d), (wih1t, wih1_d),
                       (whh1t, whh1_d), (bias0c, b0_d), (bias1c, b1g_d),
                       (w1t, w1t_d), (b1c, b1_d)):
            nc.sync.dma_start(out=sb, in_=dr)
        nc.gpsimd.dma_start(
            out=wtsr, in_=_ext(wts_d, [[0, BL], wts_d.ap[0], wts_d.ap[1]]))
        nc.vector.memset(ones, 1.0)
        make_identity(nc, ident)

        h1_last = None

        with ExitStack() as lp:
            pool_idx = lp.enter_context(tc.tile_pool(name="pool_idx", bufs=1))
            pool_x = lp.enter_context(tc.tile_pool(name="pool_x", bufs=1))
            gpp = lp.enter_context(tc.tile_pool(name="gpp", bufs=3))
            gcp = lp.enter_context(tc.tile_pool(name="gcp", bufs=3))
            xt4p = lp.enter_context(tc.tile_pool(name="xt4p", bufs=2))
            y04p = lp.enter_context(tc.tile_pool(name="y04p", bufs=3))
            gx0p = lp.enter_context(tc.tile_pool(name="gx0p", bufs=2))
            gx1p = lp.enter_context(tc.tile_pool(name="gx1p", bufs=2))
            sigp = lp.enter_context(tc.tile_pool(name="sigp", bufs=2))
            tgp = lp.enter_context(tc.tile_pool(name="tgp", bufs=2))
            tcp = lp.enter_context(tc.tile_pool(name="tcp", bufs=2))
            cp0 = lp.enter_context(tc.tile_pool(name="cp0", bufs=2))
            cp1 = lp.enter_context(tc.tile_pool(name="cp1", bufs=2))
            tmpp = lp.enter_context(tc.tile_pool(name="tmpp", bufs=4))
            ufp = lp.enter_context(tc.tile_pool(name="ufp", bufs=1))
            ppg0 = lp.enter_context(tc.tile_pool(name="ppg0", bufs=2, space="PSUM"))
            ppg1 = lp.enter_context(tc.tile_pool(name="ppg1", bufs=2, space="PSUM"))
            ppxt = lp.enter_context(tc.tile_pool(name="ppxt", bufs=2, space="PSUM"))
            ppgx = lp.enter_context(tc.tile_pool(name="ppgx", bufs=2, space="PSUM"))

            pidx = pool_idx.tile([BL, S * L], I32)
            cidx = pool_idx.tile([BL, S * L], I32)
            tss = pool_idx.tile([BL, S], F32)
            agt = pool_idx.tile([BL, 2], F32)
            nc.sync.dma_start(out=pidx, in_=pidx_d)
            nc.sync.dma_start(out=cidx, in_=cidx_d)
            nc.sync.dma_start(out=tss, in_=tss_d)
            nc.sync.dma_start(out=agt, in_=ag_d)

            xall = pool_x.tile([BL, S, IN], F32)

            # ---- ts features: x[:, :, 96:112] = t * W_ts + b_ts ----
            # wtsr rows: 0=W_ts row, 1=b_ts, 2=W_uf[:,0], 3=W_uf[:,1], 4=b_uf
            def _rowbc3(row, mid):
                r = wtsr[:, row, :]
                return _ext(r, [r.ap[0], [0, mid], r.ap[-1]])

            tss3 = _ext(tss[:], [tss.ap[0], tss.ap[1], [0, TD]])
            xts = xall[:, :, PD + CD:PD + CD + TD]
            nc.vector.tensor_tensor(out=xts, in0=tss3, in1=_rowbc3(0, S), op=ALU.mult)
            nc.vector.tensor_tensor(out=xts, in0=xts, in1=_rowbc3(1, S), op=ALU.add)

            # ---- user features: uf = age*W_uf[:,0] + gender*W_uf[:,1] + b_uf ----
            uft = ufp.tile([BL, UD], F32)
            nc.vector.scalar_tensor_tensor(
                out=uft, in0=wtsr[:, 2, :], scalar=agt[:, 0:1],
                in1=wtsr[:, 4, :], op0=ALU.mult, op1=ALU.add)
            nc.vector.scalar_tensor_tensor(
                out=uft, in0=wtsr[:, 3, :], scalar=agt[:, 1:2],
                in1=uft, op0=ALU.mult, op1=ALU.add)
            ufbc = _ext(uft[:], [uft.ap[0], [0, S], uft.ap[-1]])
            nc.vector.tensor_copy(out=xall[:, :, PD + CD + TD:], in_=ufbc)

            # ---- embedding gathers + L-sum (mean folded into W_ih0) ----
            TOK = NSB * L  # indices per partition per block
            for k in range(NGB):
                gp = gpp.tile([BL, TOK * PD], TAB_DT)
                for t in range(TOK):
                    nc.gpsimd.indirect_dma_start(
                        out=gp[:, t * PD:(t + 1) * PD], out_offset=None,
                        in_=embp_d,
                        in_offset=IndirectOffsetOnAxis(
                            ap=pidx[:, k * TOK + t:k * TOK + t + 1], axis=0))
                nc.vector.tensor_reduce(
                    out=xall[:, k * NSB:(k + 1) * NSB, 0:PD],
                    in_=gp.rearrange("p (s l d) -> p s d l", s=NSB, l=L, d=PD),
                    axis=mybir.AxisListType.X, op=ALU.add)
                gc = gcp.tile([BL, TOK * CD], TAB_DT)
                for t in range(TOK):
                    nc.gpsimd.indirect_dma_start(
                        out=gc[:, t * CD:(t + 1) * CD], out_offset=None,
                        in_=embc_d,
                        in_offset=IndirectOffsetOnAxis(
                            ap=cidx[:, k * TOK + t:k * TOK + t + 1], axis=0))
                nc.vector.tensor_reduce(
                    out=xall[:, k * NSB:(k + 1) * NSB, PD:PD + CD],
                    in_=gc.rearrange("p (s l d) -> p s d l", s=NSB, l=L, d=CD),
                    axis=mybir.AxisListType.X, op=ALU.add)

            # ---- 2-layer LSTM, feature-major ----
            def cell(gates_src, sig_n, c_prev, cpool, h_out):
                """gates_src: [128, 512] (i,f,o,g pre-activations, bias folded).
                Returns c_new; writes h into h_out."""
                sig = sigp.tile([HID, 3 * HID], F32, name=f"sig{sig_n}", tag="sig")
                nc.scalar.activation(sig, gates_src[:, 0:3 * HID], AF.Sigmoid)
                tg = tgp.tile([HID, HID], F32, name=f"tg{sig_n}", tag="tg")
                nc.scalar.activation(tg, gates_src[:, 3 * HID:], AF.Tanh)
                c_new = cpool.tile([HID, HID], F32, name=f"c{sig_n}", tag="c")
                if c_prev is None:
                    nc.vector.tensor_mul(c_new, sig[:, 0:HID], tg)
                else:
                    m1 = tmpp.tile([HID, HID], F32, name=f"m1_{sig_n}", tag="tmp")
                    nc.vector.tensor_mul(m1, sig[:, HID:2 * HID], c_prev)
                    m2 = tmpp.tile([HID, HID], F32, name=f"m2_{sig_n}", tag="tmp")
                    nc.vector.tensor_mul(m2, sig[:, 0:HID], tg)
                    nc.vector.tensor_add(c_new, m1, m2)
                tch = tcp.tile([HID, HID], F32, name=f"tc{sig_n}", tag="tc")
                nc.scalar.activation(tch, c_new, AF.Tanh)
                nc.vector.tensor_mul(h_out, sig[:, 2 * HID:], tch)
                return c_new

            c0 = c1 = None
            h1_prev = None
            y04_prev = None
            for s0 in range(0, S, GRP):
                gs = min(GRP, S - s0)
                # transpose x_s for the group: PSUM <- X[:, s, :].T
                pxt = ppxt.tile([IN, gs * BL], F32)
                for sl in range(gs):
                    nc.tensor.transpose(pxt[:, sl * BL:(sl + 1) * BL],
                                        xall[:, s0 + sl, :], ident)
                xt4 = xt4p.tile([IN, gs * BL], W_DT)
                nc.vector.tensor_copy(xt4, pxt)
                # layer-0 x-part for the group: gx0[s] = W_ih0 @ x_s^T (+bias)
                gx0 = gx0p.tile([HID, gs, 4 * HID], F32)
                for g in range(4):
                    pgx = ppgx.tile([HID, gs * BL], F32, name="pgx0", tag="pgx")
                    nc.tensor.matmul(pgx, lhsT=wih0t[:, g * HID:(g + 1) * HID],
                                     rhs=xt4, start=True, stop=True)
                    nc.scalar.activation(
                        gx0[:, :, g * HID:(g + 1) * HID],
                        pgx.rearrange("p (s b) -> p s b", s=gs),
                        AF.Identity, bias=bias0c[:, g:g + 1])
                y04 = y04p.tile([HID, gs, BL], W_DT)
                for sl in range(gs):
                    s = s0 + sl
                    if s == 0:
                        gates = gx0[:, 0, :]
                    else:
                        h_prev = (y04[:, sl - 1, :] if sl > 0
                                  else y04_prev[:, y04_prev.shape[1] - 1, :])
                        pg = ppg0.tile([HID, 4 * HID], F32, name="pg0")
                        for g in range(4):
                            nc.tensor.matmul(pg[:, g * HID:(g + 1) * HID],
                                             lhsT=whh0t[:, g * HID:(g + 1) * HID],
                                             rhs=h_prev, start=True, stop=True)
                        nc.vector.tensor_add(pg, pg, gx0[:, sl, :])
                        gates = pg
                    c0 = cell(gates, f"0_{s}", c0 if s > 0 else None, cp0,
                              y04[:, sl, :])
                # layer-1 x-part for the group
                gx1 = gx1p.tile([HID, gs, 4 * HID], F32)
                for g in range(4):
                    pgx = ppgx.tile([HID, gs * BL], F32, name="pgx1", tag="pgx")
                    nc.tensor.matmul(pgx, lhsT=wih1t[:, g * HID:(g + 1) * HID],
                                     rhs=y04.rearrange("p s b -> p (s b)"),
                                     start=True, stop=True)
                    nc.scalar.activation(
                        gx1[:, :, g * HID:(g + 1) * HID],
                        pgx.rearrange("p (s b) -> p s b", s=gs),
                        AF.Identity, bias=bias1c[:, g:g + 1])
                if debug_taps:
                    nc.sync.dma_start(out=y0_t[s0:s0 + gs], in_=y04)
                for sl in range(gs):
                    s = s0 + sl
                    if s == 0:
                        gates = gx1[:, 0, :]
                    else:
                        pg = ppg1.tile([HID, 4 * HID], F32, name="pg1")
                        for g in range(4):
                            nc.tensor.matmul(pg[:, g * HID:(g + 1) * HID],
                                             lhsT=whh1t[:, g * HID:(g + 1) * HID],
                                             rhs=h1_prev, start=True, stop=True)
                        nc.vector.tensor_add(pg, pg, gx1[:, sl, :])
                        gates = pg
                    h1_new = h1p.tile([HID, HID], W_DT, name="h1", tag="h1")
                    c1 = cell(gates, f"1_{s}", c1 if s > 0 else None, cp1, h1_new)
                    h1_prev = h1_new
                y04_prev = y04
            h1_last = h1_prev
            if debug_taps:
                nc.sync.dma_start(out=xall_t, in_=xall)
                nc.sync.dma_start(out=h1_t, in_=h1_last)

        # ---- head: hidden = relu(W1 @ h_last^T + b1); logits tiles ----
        with ExitStack() as hp:
            w2pool = hp.enter_context(tc.tile_pool(name="w2pool", bufs=3))
            outpool = hp.enter_context(tc.tile_pool(name="outpool", bufs=2))
            hidpool = hp.enter_context(tc.tile_pool(name="hidpool", bufs=1))
            b2chp = hp.enter_context(tc.tile_pool(name="b2chp", bufs=2))
            plg = hp.enter_context(tc.tile_pool(name="plg", bufs=6, space="PSUM"))
            phid_p = hp.enter_context(tc.tile_pool(name="phid_p", bufs=1, space="PSUM"))

            phid = phid_p.tile([HID // 2, BL], F32)
            nc.tensor.matmul(phid, lhsT=w1t, rhs=h1_last, start=True, stop=True)
            # hidden duplicated into both partition halves so each half-tile
            # matmul reads lhsT/rhs from the same base partition
            hid = hidpool.tile([HID, BL], W2_DT)
            nc.scalar.activation(hid[0:HID // 2, :], phid, AF.Relu, bias=b1c)
            nc.scalar.activation(hid[HID // 2:, :], phid, AF.Relu, bias=b1c)
            if debug_taps:
                nc.sync.dma_start(out=hid_t, in_=hid)

            for ch in range(NCHUNK):
                wch = w2pool.tile([128, CP * VTILE], W2_DT)
                nc.sync.dma_start(
                    out=wch, in_=w2s_d[:, ch * CP * VTILE:(ch + 1) * CP * VTILE])
                b2ch = b2chp.tile([1, CP * 2 * VTILE], F32, name="b2ch", tag="b2ch")
                nc.sync.dma_start(
                    out=b2ch,
                    in_=b2r_d[:, ch * CP * 2 * VTILE:(ch + 1) * CP * 2 * VTILE])
                och = outpool.tile([BL, CP * 2 * VTILE], OUT_DT)
                for j in range(CP):
                    pair = ch * CP + j
                    for half in range(2):
                        t = 2 * pair + half
                        pt = plg.tile([BL, VTILE], F32, name="pt")
                        # b2 added via a K=1 matmul of a ones row against the
                        # bias slice (both at partition t%128), then the main
                        # K=64 matmul accumulates on top.
                        nc.tensor.matmul(
                            pt, lhsT=ones,
                            rhs=b2ch[:, (2 * j + half) * VTILE:
                                     (2 * j + half + 1) * VTILE],
                            start=True, stop=False, skip_group_check=True)
                        nc.tensor.matmul(
                            pt, lhsT=hid[64 * half:64 * (half + 1), :],
                            rhs=wch[64 * half:64 * (half + 1),
                                    j * VTILE:(j + 1) * VTILE],
                            start=False, stop=True, skip_group_check=True)
                        osl = och[:, (2 * j + half) * VTILE:
                                  (2 * j + half + 1) * VTILE]
                        if half == 0:
                            nc.vector.tensor_copy(out=osl, in_=pt)
                        else:
                            nc.scalar.copy(out=osl, in_=pt)
                nc.sync.dma_start(
                    out=out_d[:, ch * CP * 2 * VTILE:(ch + 1) * CP * 2 * VTILE],
                    in_=och)

    nc.compile()
    return nc


# ---------------- host-side preparation ----------------

def _np(x, dt=np.float32):
    return np.ascontiguousarray(np.asarray(x), dtype=dt)


def _perm_gates(w):
    """torch gate order (i,f,g,o) rows -> (i,f,o,g)."""
    H = HID
    return np.concatenate([w[0:H], w[H:2 * H], w[3 * H:4 * H], w[2 * H:3 * H]], 0)


def prep_shared(inp):
    """Build the shared (weight) arrays for every core."""
    td = mybir.dt.np(TAB_DT)
    wd = mybir.dt.np(W_DT)
    w2d = mybir.dt.np(W2_DT)

    wih0 = _np(inp["W_ih0"]).copy()
    wih0[:, 0:PD + CD] /= L  # fold the basket mean
    d = {
        "embp": _np(inp["emb_p"], td),
        "embc": _np(inp["emb_c"], td),
        "wih0t": np.ascontiguousarray(_perm_gates(wih0).T, wd),
        "whh0t": np.ascontiguousarray(_perm_gates(_np(inp["W_hh0"])).T, wd),
        "wih1t": np.ascontiguousarray(_perm_gates(_np(inp["W_ih1"])).T, wd),
        "whh1t": np.ascontiguousarray(_perm_gates(_np(inp["W_hh1"])).T, wd),
        "bias0c": np.ascontiguousarray(
            _perm_gates(_np(inp["b_ih0"]) + _np(inp["b_hh0"])).reshape(4, HID).T,
            np.float32),
        "bias1c": np.ascontiguousarray(
            _perm_gates(_np(inp["b_ih1"]) + _np(inp["b_hh1"])).reshape(4, HID).T,
            np.float32),
        "w1t": np.ascontiguousarray(_np(inp["W1"]).T, wd),
        "b1c": _np(inp["b1"]).reshape(HID // 2, 1),
    }
    wts = np.zeros((5, TD), np.float32)
    wts[0] = _np(inp["W_ts"]).reshape(TD)
    wts[1] = _np(inp["b_ts"])
    wts[2] = _np(inp["W_uf"])[:, 0]
    wts[3] = _np(inp["W_uf"])[:, 1]
    wts[4] = _np(inp["b_uf"])
    d["wtsrows"] = wts

    w2t = np.zeros((HID // 2, VP), np.float32)
    w2t[:, :NPROD] = _np(inp["W2"]).T
    w2r = w2t.reshape(HID // 2, NT // 2, 2, VTILE)
    d["w2s"] = np.ascontiguousarray(
        np.concatenate([w2r[:, :, 0, :], w2r[:, :, 1, :]], axis=0)
        .reshape(128, NPAIR * VTILE), w2d)

    b2p = np.zeros(VP, np.float32)
    b2p[:NPROD] = _np(inp["b2"])
    d["b2p"] = b2p.reshape(1, VP)
    return d


def core_inputs(inp, shared, k):
    lo, hi = k * BL, (k + 1) * BL
    d = dict(shared)
    d["pidx"] = _np(inp["product_input"], np.int32)[lo:hi].reshape(BL, S * L)
    d["cidx"] = _np(inp["categories_input"], np.int32)[lo:hi].reshape(BL, S * L)
    d["tss"] = _np(inp["user_timestamps_input"])[lo:hi]
    d["ag"] = np.ascontiguousarray(
        np.stack([_np(inp["user_age_input"])[lo:hi],
                  _np(inp["user_gender_input"])[lo:hi]], axis=1))
    return d


_NC_CACHE = None


def get_nc():
    global _NC_CACHE
    if _NC_CACHE is None:
        _NC_CACHE = build_nc()
    return _NC_CACHE


def kernel(**inputs):
    from concourse.bass_utils import run_bass_kernel_spmd
    nc = get_nc()
    shared = prep_shared(inputs)
    in_maps = [core_inputs(inputs, shared, k) for k in range(NCORES)]
    res = run_bass_kernel_spmd(nc, in_maps, core_ids=list(range(NCORES)))
    out = np.concatenate([r["logits"][:, :NPROD] for r in res.results], axis=0)
    return out.astype(np.float32)

